# revision 21
# baseline (speedup 1.0000x reference)
"""Causal multi-head attention on 8 trn2 NeuronCores.

Problem: B=2, S=2048, D=2048, H=16 (HD=128), fp32 in/out.
Sharding: tensor-parallel over heads - core c owns heads {2c, 2c+1} for both
batches. Each core computes its Q/K/V projections, attention for its 4
(batch, head) pairs, and a partial output projection over its head slice.
The host sums the 8 partial outputs (transposing [B,D,S] -> [B,S,D]) and
adds the output bias.

All operands are bf16 in SBUF (fp32 PSUM accumulation), which keeps Q/K/V
fully SBUF-resident (no DRAM spill) and runs every matmul at 1 cycle/row.

Device algorithm (per core):
  Phase A: stream X^T in 2KB-line DMAs, compute Q^T/K^T (head-dim on
           partitions) and V (tokens on partitions), all SBUF-resident.
           V gets a ones-column appended ([V | 1], width HD+1).
  Phase B: per (b, qb of 512 queries): score tiles S^T = K^T_chunk.T @ Q^T
           (k on partitions), E = exp(S^T/sqrt(hd)) in bf16 with causal 0/1
           mask multiplies (Pool engine) on diagonal tiles. Then per
           128-query chunk i: ctx_ext[q, 0:129] = sum_j E_chunk(j).T @ [V|1]
           accumulated in PSUM - column 128 is the softmax denominator for
           free. A [128,1] DVE reciprocal + DVE per-partition-scalar multiply
           normalizes ctx into bf16, and a PE transpose flips it to [hd, q].
  Phase C: out^T tiles = sum_h Wo_chunk.T @ ctx^T, written as OUT[b, D, S].

  ACT exp (~650ns/tile) is slower than the 213ns score matmul, so score ops
  are WOVEN into the rest of the PE stream (projection chains of batch 1,
  AV and out-projection matmuls of earlier groups) at one per ~650ns of PE
  time, with a 2-group lookahead so every exp has runway; the first two
  groups' scores hide inside phase A's batch-1 projections.
  No max-subtraction is needed: scores are O(5) for this problem so exp
  cannot overflow, and softmax is shift-invariant.
"""

import os
from collections import deque

import numpy as np
import ml_dtypes

import concourse.bacc as bacc
import concourse.tile as tile
from concourse import mybir
from concourse.bass_utils import run_bass_kernel_spmd

BF16 = ml_dtypes.bfloat16


def _install_neff_cache():
    """Cache compiled NEFFs on disk keyed by BIR content hash.

    Purely a compile-time memo: identical BIR -> identical NEFF, so repeat
    runs skip the multi-minute neuronxcc compile. No effect on execution.
    """
    import hashlib
    import shutil

    import concourse.bass2jax as _b2j
    import concourse.bass_utils as _bu

    if getattr(_bu, "_neff_cache_installed", False):
        return
    cache_dir = os.environ.get("NEFF_CACHE_DIR", "/tmp/neff_cache")
    orig = _bu.compile_bir_kernel

    def cached(bir_json, tmpdir, neff_name="file.neff"):
        try:
            os.makedirs(cache_dir, exist_ok=True)
            key = hashlib.sha256(bir_json).hexdigest()[:24]
            cpath = os.path.join(cache_dir, key + ".neff")
            dst = os.path.join(tmpdir, neff_name)
            if os.path.exists(cpath):
                shutil.copy(cpath, dst)
                return dst
            out = orig(bir_json, tmpdir, neff_name)
            shutil.copy(out, cpath)
            return out
        except OSError:
            return orig(bir_json, tmpdir, neff_name)

    _bu.compile_bir_kernel = cached
    _b2j.compile_bir_kernel = cached
    _bu._neff_cache_installed = True


_install_neff_cache()

B, S, D, H = 2, 2048, 2048, 16
HD = D // H          # 128
NCORES = 8
HPC = H // NCORES    # heads per core = 2
M = HPC * HD         # 256 output columns per core per projection
T = B * S            # 4096 total token rows
KO = D // 128        # 16 contraction chunks
NPAIR = T // 1024    # 4 phase-A token pairs of 1024
QB = S // 512        # 4 query blocks per batch
SC = S // 128        # 16 key chunks per sequence
HD1 = HD + 1         # V with ones column
SCALE = 1.0 / float(np.sqrt(HD))

_built = {}


def _build(with_bias):
    f32 = mybir.dt.float32
    bf16 = mybir.dt.bfloat16
    AF = mybir.ActivationFunctionType

    nc = bacc.Bacc(None, target_bir_lowering=False)

    # ---- per-core DRAM parameters (host supplies per-core shards) ----
    # XT[p, pair, ko, t] = x[pair*1024 + t, ko*128 + p]
    xt_p = nc.declare_dram_parameter("XT", [128, NPAIR, KO, 1024], bf16, False)
    # WqT/WkT/WvT[p, ko, m] = W[rows0 + m, ko*128 + p]
    wqt_p = nc.declare_dram_parameter("WQT", [128, KO, M], bf16, False)
    wkt_p = nc.declare_dram_parameter("WKT", [128, KO, M], bf16, False)
    wvt_p = nc.declare_dram_parameter("WVT", [128, KO, M], bf16, False)
    # WOT[p, h, oc, j] = Wo[oc*128 + j, rows0 + h*128 + p]
    wot_p = nc.declare_dram_parameter("WOT", [128, HPC, KO, 128], bf16, False)
    bias_p = nc.declare_dram_parameter("BIAS", [1, 3, M], bf16, False)
    mask_p = nc.declare_dram_parameter("MASK", [128, 4, 512], bf16, False)
    iden_p = nc.declare_dram_parameter("IDEN", [128, 128], bf16, False)
    ones_p = nc.declare_dram_parameter("ONES", [128, 512], bf16, False)
    out_p = nc.declare_dram_parameter("OUT", [B, D, S], f32, True)

    groups = [(b, qb) for b in range(B) for qb in range(QB)]
    SPACING = 600.0

    with tile.TileContext(nc) as tc:
        with (
            tc.tile_pool(name="persist", bufs=1) as persist,
            tc.tile_pool(name="bconst", bufs=1) as bconst,
            tc.tile_pool(name="epool", bufs=64) as epool,
        ):
            qt_res = persist.tile([128, B, HPC, S], bf16)
            kt_res = persist.tile([128, B, HPC, S], bf16)
            v_res = persist.tile([128, B, HPC, SC, HD1], bf16)
            # ones column of [V | 1]; disjoint from the phase-A V writes
            nc.vector.memset(v_res[:, :, :, :, HD:HD1], 1.0)

            # phase-B/C constants (DMAs queued below, after pair-0's X)
            masks = bconst.tile([128, 4, 512], bf16, tag="masks")
            wot = bconst.tile([128, HPC, KO, 128], bf16, tag="wot")
            iden = bconst.tile([128, 128], bf16, tag="iden")

            def make_score_op(pool, tag, b, qb, t, h, out_list):
                def fn():
                    pss = pool.tile([128, 512], f32, tag=tag, name="pss")
                    nc.tensor.matmul(
                        pss,
                        lhsT=kt_res[:, b, h, t * 128 : (t + 1) * 128],
                        rhs=qt_res[:, b, h, qb * 512 : (qb + 1) * 512],
                        start=True,
                        stop=True,
                    )
                    e = epool.tile([128, 512], bf16, tag="e", name="e")
                    nc.scalar.activation(e, pss, AF.Exp, scale=SCALE)
                    if t >= 4 * qb:
                        # masking runs on the otherwise-idle Pool engine
                        nc.gpsimd.tensor_mul(e, e, masks[:, t - 4 * qb])
                    out_list.append(e)

                return fn

            def weave(pe_ops, queue, acc):
                """Emit pe_ops, inserting one queued score per SPACING ns
                of accumulated PE time. Returns the leftover accum."""
                for cost, fn in pe_ops:
                    while queue and acc >= SPACING:
                        queue.popleft()[1]()
                        acc -= SPACING
                    fn()
                    acc += cost
                return acc

            def queue_scores(queue, pool, tag, gi, es_by):
                es_by[gi] = []
                b, qb = groups[gi]
                for t in range(4 * (qb + 1)):
                    for h in range(HPC):
                        queue.append(
                            (gi, make_score_op(pool, tag, b, qb, t, h, es_by[gi]))
                        )

            es_by = {}
            queue = deque()
            acc = 0.0

            # ---------------- Phase A: projections ----------------
            with (
                tc.tile_pool(name="wqkv", bufs=1) as wpool,
                tc.tile_pool(name="xs", bufs=3) as xpool,
                tc.tile_pool(name="psQK", bufs=6, space="PSUM") as psQK,
                tc.tile_pool(name="psV", bufs=2, space="PSUM") as psV,
            ):
                wq = wpool.tile([128, KO, M], bf16, tag="wq")
                wk = wpool.tile([128, KO, M], bf16, tag="wk")
                wv = wpool.tile([128, KO, M], bf16, tag="wv")
                if with_bias:
                    bias = wpool.tile([1, 3, M], bf16, tag="bias")
                    ones_t = wpool.tile([128, 512], bf16, tag="ones_a")
                    ones = ones_t[0:1, :]

                def qk_bias_mm(ps, bi, h):
                    nc.tensor.matmul(
                        ps,
                        lhsT=bias[:, bi, h * HD : (h + 1) * HD],
                        rhs=ones,
                        start=False,
                        stop=True,
                    )

                def pair_dma(pair, xt_h):
                    if pair == 0:
                        # JIT startup: 2-ko X chunks interleaved with wq so
                        # the four Q accumulation chains start ~2us in and
                        # stay fed; everything phase B needs comes last
                        for g in range(8):
                            xsl = slice((g % 4) * 2, (g % 4) * 2 + 2)
                            (nc.scalar if g % 2 else nc.sync).dma_start(
                                wq[:, 2 * g : 2 * g + 2],
                                wqt_p[:, 2 * g : 2 * g + 2],
                            )
                            (nc.sync if g % 2 else nc.scalar).dma_start(
                                xt_h[g // 4][:, xsl],
                                xt_p[:, 0, 2 * g : 2 * g + 2],
                            )
                        nc.sync.dma_start(wk, wkt_p[:])
                        nc.sync.dma_start(wv, wvt_p[:])
                        nc.sync.dma_start(masks, mask_p[:])
                        nc.sync.dma_start(wot, wot_p[:])
                        nc.sync.dma_start(iden, iden_p[:])
                        if with_bias:
                            nc.sync.dma_start(bias, bias_p[:])
                            nc.sync.dma_start(ones_t, ones_p[:])
                    else:
                        for half in range(2):
                            nc.sync.dma_start(
                                xt_h[half],
                                xt_p[:, pair, half * 8 : half * 8 + 8],
                            )

                def pair_ops(pair, xt_h):
                    """Projection compute for one 1024-token pair as a list
                    of (cost_ns, fn) ops at single-ko granularity."""
                    b = pair // 2
                    state = {}

                    def xt_at(ko, sub):
                        return xt_h[ko // 8][
                            :, ko % 8, sub * 512 : (sub + 1) * 512
                        ]

                    ops = []
                    for sub in range(2):
                        s0 = (pair * 1024 + sub * 512) % S
                        for wt, dst, bi in ((wq, qt_res, 0), (wk, kt_res, 1)):
                            for h in range(HPC):
                                for ko in range(KO):
                                    def fn(
                                        sub=sub, s0=s0, wt=wt, dst=dst,
                                        bi=bi, h=h, ko=ko,
                                    ):
                                        key = (sub, bi, h)
                                        if ko == 0:
                                            state[key] = psQK.tile(
                                                [128, 512], f32,
                                                tag="qk", name="psqk",
                                            )
                                        ps = state[key]
                                        nc.tensor.matmul(
                                            ps,
                                            lhsT=wt[:, ko, h * HD : (h + 1) * HD],
                                            rhs=xt_at(ko, sub),
                                            start=(ko == 0),
                                            stop=(ko == KO - 1)
                                            and not with_bias,
                                        )
                                        if ko == KO - 1:
                                            if with_bias:
                                                qk_bias_mm(ps, bi, h)
                                            nc.vector.tensor_copy(
                                                dst[:, b, h, s0 : s0 + 512], ps
                                            )
                                    ops.append((213, fn))
                        for tsub in range(4):
                            for ko in range(KO):
                                def fn(sub=sub, s0=s0, tsub=tsub, ko=ko):
                                    key = ("v", sub, tsub)
                                    if ko == 0:
                                        state[key] = psV.tile(
                                            [128, M], f32, tag="v", name="psv"
                                        )
                                    ps = state[key]
                                    nc.tensor.matmul(
                                        ps,
                                        lhsT=xt_at(ko, sub)[
                                            :, tsub * 128 : (tsub + 1) * 128
                                        ],
                                        rhs=wv[:, ko],
                                        start=(ko == 0),
                                        stop=(ko == KO - 1) and not with_bias,
                                    )
                                    if ko == KO - 1:
                                        if with_bias:
                                            nc.tensor.matmul(
                                                ps,
                                                lhsT=ones[:, :128],
                                                rhs=bias[:, 2],
                                                start=False,
                                                stop=True,
                                            )
                                        sc = (s0 + tsub * 128) // 128
                                        nc.vector.tensor_copy(
                                            v_res[:, b, :, sc, 0:HD],
                                            ps.rearrange(
                                                "p (h d) -> p h d", h=HPC
                                            ),
                                        )
                                ops.append((107, fn))
                    return ops

                # pair 0: special interleaved-chain startup, emitted bare
                xt_h0 = [
                    xpool.tile([128, KO // 2, 1024], bf16, tag="xt", name="xth")
                    for _ in range(2)
                ]
                pair_dma(0, xt_h0)

                def xt_at0(ko, sub):
                    return xt_h0[ko // 8][:, ko % 8, sub * 512 : (sub + 1) * 512]

                chains = [(h, sub) for h in range(HPC) for sub in range(2)]
                ps_q = {
                    c: psQK.tile([128, 512], f32, tag="qk", name="psq")
                    for c in chains
                }
                for ko in range(KO):
                    for h, sub in chains:
                        nc.tensor.matmul(
                            ps_q[(h, sub)],
                            lhsT=wq[:, ko, h * HD : (h + 1) * HD],
                            rhs=xt_at0(ko, sub),
                            start=(ko == 0),
                            stop=(ko == KO - 1) and not with_bias,
                        )
                for h, sub in chains:
                    if with_bias:
                        qk_bias_mm(ps_q[(h, sub)], 0, h)
                    nc.vector.tensor_copy(
                        qt_res[:, 0, h, sub * 512 : sub * 512 + 512],
                        ps_q[(h, sub)],
                    )
                # pair 0 K + V (reuse pair_ops minus the already-done Q):
                # per-sub layout is Q(2x16) K(2x16) V(4x16) = 128 ops
                ops0 = pair_ops(0, xt_h0)
                for _, fn in ops0[32:128]:      # K + V of sub 0
                    fn()
                for _, fn in ops0[160:256]:     # K + V of sub 1
                    fn()

                # pair 1: emitted bare
                xt_h1 = [
                    xpool.tile([128, KO // 2, 1024], bf16, tag="xt", name="xth")
                    for _ in range(2)
                ]
                pair_dma(1, xt_h1)
                for _, fn in pair_ops(1, xt_h1):
                    fn()

                # batch 0's Q/K/V are ready: weave groups 0+1's scores into
                # batch 1's projection compute (exps run during phase A)
                queue_scores(queue, psQK, "qk", 0, es_by)
                queue_scores(queue, psQK, "qk", 1, es_by)
                for pair in (2, 3):
                    xt_h = [
                        xpool.tile(
                            [128, KO // 2, 1024], bf16, tag="xt", name="xth"
                        )
                        for _ in range(2)
                    ]
                    pair_dma(pair, xt_h)
                    acc = weave(pair_ops(pair, xt_h), queue, acc)

            # ------------- Phase B + C: attention + out projection -------------
            with (
                tc.tile_pool(name="ctxn", bufs=12) as ctxn,
                tc.tile_pool(name="recp", bufs=12) as recp,
                tc.tile_pool(name="ctxT", bufs=2) as ctxTp,
                tc.tile_pool(name="ob", bufs=3) as obp,
                tc.tile_pool(name="psS", bufs=2, space="PSUM") as psS,
                tc.tile_pool(name="psC", bufs=3, space="PSUM") as psC,
                tc.tile_pool(name="psT", bufs=1, space="PSUM") as psT,
                tc.tile_pool(name="psO", bufs=2, space="PSUM") as psO,
            ):
                def av_ops(b, qb, es, cns_out, tc_i=None):
                    """One op per k-chunk j of each 128-query chunk i; the
                    closing op of each i-chunk adds the DVE rec+normalize
                    (plus, for the last group, its transposes via tc_i)."""
                    ops = []
                    state = {}
                    for i in range(4):
                        qi = 4 * qb + i
                        for j in range(qi + 1):
                            def fn(i=i, j=j, qi=qi):
                                if j == 0:
                                    state[i] = [
                                        psC.tile(
                                            [128, 512], f32, tag="c", name="psc"
                                        )
                                        for _ in range(HPC)
                                    ]
                                pscs = state[i]
                                for h in range(HPC):
                                    nc.tensor.matmul(
                                        pscs[h][:, 0:HD1],
                                        lhsT=es[2 * j + h][
                                            :, i * 128 : (i + 1) * 128
                                        ],
                                        rhs=v_res[:, b, h, j, :],
                                        start=(j == 0),
                                        stop=(j == qi),
                                    )
                                if j == qi:
                                    cns_pair = []
                                    for h in range(HPC):
                                        rec = recp.tile(
                                            [128, 1], f32, tag="r", name="rec"
                                        )
                                        nc.vector.reciprocal(
                                            rec, pscs[h][:, HD:HD1]
                                        )
                                        cn = ctxn.tile(
                                            [128, 128], bf16, tag="cn", name="cn"
                                        )
                                        nc.vector.tensor_scalar_mul(
                                            cn, pscs[h][:, 0:HD], rec
                                        )
                                        cns_pair.append(cn)
                                    cns_out.extend(cns_pair)
                                    if tc_i is not None:
                                        tc_i(i, cns_pair)
                            ops.append((110, fn))
                    return ops

                def make_tc_t(ct):
                    """Per-chunk transposes for the final group, so the
                    epilogue is only the out projection."""
                    def tc_i(i, cns_pair):
                        for h in range(HPC):
                            pst = psT.tile([128, 512], bf16, tag="t", name="pst")
                            nc.tensor.transpose(pst[:, 0:128], cns_pair[h], iden)
                            nc.vector.tensor_copy(
                                ct[:, h, i * 128 : (i + 1) * 128], pst[:, 0:128]
                            )
                    return tc_i

                def tc_ops(b, qb, cns, ct, skip_T=False, alt_q=False):
                    """Transpose normalized ctx, then the out projection.
                    Output tiles are paired into one DMA per 256 rows."""
                    ops = []
                    if not skip_T:
                        for i in range(4):
                            for h in range(HPC):
                                def fn(i=i, h=h):
                                    pst = psT.tile(
                                        [128, 512], bf16, tag="t", name="pst"
                                    )
                                    nc.tensor.transpose(
                                        pst[:, 0:128], cns[2 * i + h], iden
                                    )
                                    nc.vector.tensor_copy(
                                        ct[:, h, i * 128 : (i + 1) * 128],
                                        pst[:, 0:128],
                                    )
                                ops.append((110, fn))
                    state = {}
                    for oc in range(KO):
                        def fn(oc=oc):
                            pso = psO.tile([128, 512], f32, tag="o", name="pso")
                            for h in range(HPC):
                                nc.tensor.matmul(
                                    pso,
                                    lhsT=wot[:, h, oc],
                                    rhs=ct[:, h, :],
                                    start=(h == 0),
                                    stop=(h == HPC - 1),
                                )
                            if oc % 2 == 0:
                                state["ob"] = obp.tile(
                                    [128, 2, 512], f32, tag="ob", name="ob"
                                )
                                nc.vector.tensor_copy(state["ob"][:, 0], pso)
                            else:
                                ob = state["ob"]
                                nc.vector.tensor_copy(ob[:, 1], pso)
                                eng = (
                                    nc.scalar
                                    if alt_q and (oc // 2) % 2
                                    else nc.sync
                                )
                                eng.dma_start(
                                    out_p[
                                        b,
                                        (oc - 1) * 128 : (oc + 1) * 128,
                                        qb * 512 : (qb + 1) * 512,
                                    ].rearrange("(u p) s -> p u s", u=2),
                                    ob,
                                )
                        ops.append((430, fn))
                    return ops

                prev = None
                last_tc = None
                for gi, (b, qb) in enumerate(groups):
                    if gi + 2 < len(groups):
                        queue_scores(queue, psS, "s", gi + 2, es_by)
                    pre_ops = tc_ops(*prev) if prev is not None else []
                    acc = weave(pre_ops, queue, acc)
                    # barrier: scores(g) must all be emitted before AV(g)
                    while queue and queue[0][0] <= gi:
                        queue.popleft()[1]()
                        acc = 0.0
                    cns = []
                    ct = ctxTp.tile([128, HPC, 512], bf16, tag="ct", name="ct")
                    last = gi == len(groups) - 1
                    tci = make_tc_t(ct) if last else None
                    acc = weave(
                        av_ops(b, qb, es_by[gi], cns, tc_i=tci), queue, acc
                    )
                    if last:
                        last_tc = (b, qb, cns, ct)
                        prev = None
                    else:
                        prev = (b, qb, cns, ct)
                    del es_by[gi]
                if prev is not None:
                    for _, fn in tc_ops(*prev):
                        fn()
                if last_tc is not None:
                    for _, fn in tc_ops(*last_tc, skip_T=True, alt_q=True):
                        fn()

    nc.finalize()
    return nc


def _get_nc(with_bias=False):
    if with_bias not in _built:
        _built[with_bias] = _build(with_bias)
    return _built[with_bias]


def kernel(hidden_states, attention_mask, Wq, bq, Wk, bk, Wv, bv, Wo, bo):
    hidden_states = np.asarray(hidden_states, dtype=np.float32)
    Wq, Wk, Wv, Wo = (np.asarray(w, dtype=np.float32) for w in (Wq, Wk, Wv, Wo))
    bq, bk, bv, bo = (np.asarray(v, dtype=np.float32) for v in (bq, bk, bv, bo))

    with_bias = bool(np.any(bq) or np.any(bk) or np.any(bv))

    x = hidden_states.reshape(T, D)
    # XT[p, pair, ko, t] = x[pair*1024 + t, ko*128 + p]
    xt = np.ascontiguousarray(
        x.reshape(NPAIR, 1024, KO, 128).transpose(3, 0, 2, 1)
    ).astype(BF16)

    # causal 0/1 masks for the 4 diagonal-tile offsets:
    # mask[p, i, f] = p + 128*i <= f
    p_idx = np.arange(128)[:, None, None]
    i_idx = np.arange(4)[None, :, None]
    f_idx = np.arange(512)[None, None, :]
    mask = (p_idx + 128 * i_idx <= f_idx).astype(BF16)
    iden = np.eye(128, dtype=BF16)
    ones = np.ones((128, 512), dtype=BF16)

    in_maps = []
    for c in range(NCORES):
        rows = slice(c * M, (c + 1) * M)
        # W*T[p, ko, m] = W[rows0 + m, ko*128 + p]
        wqt = np.ascontiguousarray(
            Wq[rows, :].T.reshape(KO, 128, M).transpose(1, 0, 2)
        ).astype(BF16)
        wkt = np.ascontiguousarray(
            Wk[rows, :].T.reshape(KO, 128, M).transpose(1, 0, 2)
        ).astype(BF16)
        wvt = np.ascontiguousarray(
            Wv[rows, :].T.reshape(KO, 128, M).transpose(1, 0, 2)
        ).astype(BF16)
        # WOT[p, h, oc, j] = Wo[oc*128 + j, rows0 + h*128 + p]
        wot = np.ascontiguousarray(
            Wo[:, rows].reshape(KO, 128, HPC, 128).transpose(3, 2, 0, 1)
        ).astype(BF16)
        bias = np.stack([bq[rows], bk[rows], bv[rows]])[None].astype(BF16)
        in_maps.append(
            {
                "XT": xt,
                "WQT": wqt,
                "WKT": wkt,
                "WVT": wvt,
                "WOT": wot,
                "BIAS": np.ascontiguousarray(bias),
                "MASK": mask,
                "IDEN": iden,
                "ONES": ones,
            }
        )

    res = run_bass_kernel_spmd(_get_nc(with_bias), in_maps, list(range(NCORES)))
    out = res.results[0]["OUT"].copy()
    for c in range(1, NCORES):
        out += res.results[c]["OUT"]
    out = np.ascontiguousarray(out.transpose(0, 2, 1))
    out += bo
    return out


# revision 23
# speedup vs baseline: 1.0083x; 1.0083x over previous
"""Causal multi-head attention on 8 trn2 NeuronCores.

Problem: B=2, S=2048, D=2048, H=16 (HD=128), fp32 in/out.
Sharding: tensor-parallel over heads - core c owns heads {2c, 2c+1} for both
batches. Each core computes its Q/K/V projections, attention for its 4
(batch, head) pairs, and a partial output projection over its head slice.
The host sums the 8 partial outputs (transposing [B,D,S] -> [B,S,D]) and
adds the output bias.

All operands are bf16 in SBUF (fp32 PSUM accumulation), which keeps Q/K/V
fully SBUF-resident (no DRAM spill) and runs every matmul at 1 cycle/row.

Device algorithm (per core):
  Phase A: stream X^T in 2KB-line DMAs, compute Q^T/K^T (head-dim on
           partitions) and V (tokens on partitions), all SBUF-resident.
           V gets a ones-column appended ([V | 1], width HD+1).
  Phase B: per (b, qb of 512 queries): score tiles S^T = K^T_chunk.T @ Q^T
           (k on partitions), E = exp(S^T/sqrt(hd)) in bf16 with causal 0/1
           mask multiplies (Pool engine) on diagonal tiles. Then per
           128-query chunk i: ctx_ext[q, 0:129] = sum_j E_chunk(j).T @ [V|1]
           accumulated in PSUM - column 128 is the softmax denominator for
           free. A [128,1] DVE reciprocal + DVE per-partition-scalar multiply
           normalizes ctx into bf16, and a PE transpose flips it to [hd, q].
  Phase C: out^T tiles = sum_h Wo_chunk.T @ ctx^T, written as OUT[b, D, S].

  ACT exp (~650ns/tile) is slower than the 213ns score matmul, so score ops
  are WOVEN into the rest of the PE stream (projection chains of batch 1,
  AV and out-projection matmuls of earlier groups) at one per ~650ns of PE
  time, with a 2-group lookahead so every exp has runway; the first two
  groups' scores hide inside phase A's batch-1 projections.
  No max-subtraction is needed: scores are O(5) for this problem so exp
  cannot overflow, and softmax is shift-invariant.
"""

import os
from collections import deque

import numpy as np
import ml_dtypes

import concourse.bacc as bacc
import concourse.tile as tile
from concourse import mybir
from concourse.bass_utils import run_bass_kernel_spmd

BF16 = ml_dtypes.bfloat16


def _install_neff_cache():
    """Cache compiled NEFFs on disk keyed by BIR content hash.

    Purely a compile-time memo: identical BIR -> identical NEFF, so repeat
    runs skip the multi-minute neuronxcc compile. No effect on execution.
    """
    import hashlib
    import shutil

    import concourse.bass2jax as _b2j
    import concourse.bass_utils as _bu

    if getattr(_bu, "_neff_cache_installed", False):
        return
    cache_dir = os.environ.get("NEFF_CACHE_DIR", "/tmp/neff_cache")
    orig = _bu.compile_bir_kernel

    def cached(bir_json, tmpdir, neff_name="file.neff"):
        try:
            os.makedirs(cache_dir, exist_ok=True)
            key = hashlib.sha256(bir_json).hexdigest()[:24]
            cpath = os.path.join(cache_dir, key + ".neff")
            dst = os.path.join(tmpdir, neff_name)
            if os.path.exists(cpath):
                shutil.copy(cpath, dst)
                return dst
            out = orig(bir_json, tmpdir, neff_name)
            shutil.copy(out, cpath)
            return out
        except OSError:
            return orig(bir_json, tmpdir, neff_name)

    _bu.compile_bir_kernel = cached
    _b2j.compile_bir_kernel = cached
    _bu._neff_cache_installed = True


_install_neff_cache()

B, S, D, H = 2, 2048, 2048, 16
HD = D // H          # 128
NCORES = 8
HPC = H // NCORES    # heads per core = 2
M = HPC * HD         # 256 output columns per core per projection
T = B * S            # 4096 total token rows
KO = D // 128        # 16 contraction chunks
NPAIR = T // 1024    # 4 phase-A token pairs of 1024
QB = S // 512        # 4 query blocks per batch
SC = S // 128        # 16 key chunks per sequence
HD1 = HD + 1         # V with ones column
SCALE = 1.0 / float(np.sqrt(HD))

_built = {}


def _build(with_bias):
    f32 = mybir.dt.float32
    bf16 = mybir.dt.bfloat16
    AF = mybir.ActivationFunctionType

    nc = bacc.Bacc(None, target_bir_lowering=False)

    # ---- per-core DRAM parameters (host supplies per-core shards) ----
    # XT[p, pair, ko, t] = x[pair*1024 + t, ko*128 + p]
    xt_p = nc.declare_dram_parameter("XT", [128, NPAIR, KO, 1024], bf16, False)
    # WqT/WkT/WvT[p, ko, m] = W[rows0 + m, ko*128 + p]
    wqt_p = nc.declare_dram_parameter("WQT", [128, KO, M], bf16, False)
    wkt_p = nc.declare_dram_parameter("WKT", [128, KO, M], bf16, False)
    wvt_p = nc.declare_dram_parameter("WVT", [128, KO, M], bf16, False)
    # WOT[p, h, oc, j] = Wo[oc*128 + j, rows0 + h*128 + p]
    wot_p = nc.declare_dram_parameter("WOT", [128, HPC, KO, 128], bf16, False)
    bias_p = nc.declare_dram_parameter("BIAS", [1, 3, M], bf16, False)
    mask_p = nc.declare_dram_parameter("MASK", [128, 4, 512], bf16, False)
    iden_p = nc.declare_dram_parameter("IDEN", [128, 128], bf16, False)
    ones_p = nc.declare_dram_parameter("ONES", [128, 512], bf16, False)
    out_p = nc.declare_dram_parameter("OUT", [B, D, S], f32, True)

    groups = [(b, qb) for b in range(B) for qb in range(QB)]
    SPACING = 600.0

    with tile.TileContext(nc) as tc:
        with (
            tc.tile_pool(name="persist", bufs=1) as persist,
            tc.tile_pool(name="bconst", bufs=1) as bconst,
            tc.tile_pool(name="epool", bufs=64) as epool,
        ):
            qt_res = persist.tile([128, B, HPC, S], bf16)
            kt_res = persist.tile([128, B, HPC, S], bf16)
            v_res = persist.tile([128, B, HPC, SC, HD1], bf16)
            # ones column of [V | 1]; disjoint from the phase-A V writes
            nc.vector.memset(v_res[:, :, :, :, HD:HD1], 1.0)

            # phase-B/C constants (DMAs queued below, after pair-0's X)
            masks = bconst.tile([128, 4, 512], bf16, tag="masks")
            wot = bconst.tile([128, HPC, KO, 128], bf16, tag="wot")
            iden = bconst.tile([128, 128], bf16, tag="iden")

            def make_score_op(pool, tag, b, qb, t, h, out_list):
                def fn():
                    pss = pool.tile([128, 512], f32, tag=tag, name="pss")
                    nc.tensor.matmul(
                        pss,
                        lhsT=kt_res[:, b, h, t * 128 : (t + 1) * 128],
                        rhs=qt_res[:, b, h, qb * 512 : (qb + 1) * 512],
                        start=True,
                        stop=True,
                    )
                    e = epool.tile([128, 512], bf16, tag="e", name="e")
                    nc.scalar.activation(e, pss, AF.Exp, scale=SCALE)
                    if t >= 4 * qb:
                        # masking runs on the otherwise-idle Pool engine
                        nc.gpsimd.tensor_mul(e, e, masks[:, t - 4 * qb])
                    out_list.append(e)

                return fn

            def weave(pe_ops, queue, acc):
                """Emit pe_ops, inserting one queued score per SPACING ns
                of accumulated PE time. Returns the leftover accum."""
                for cost, fn in pe_ops:
                    while queue and acc >= SPACING:
                        queue.popleft()[1]()
                        acc -= SPACING
                    fn()
                    acc += cost
                return acc

            def queue_scores(queue, pool, tag, gi, es_by):
                es_by[gi] = []
                b, qb = groups[gi]
                for t in range(4 * (qb + 1)):
                    for h in range(HPC):
                        queue.append(
                            (gi, make_score_op(pool, tag, b, qb, t, h, es_by[gi]))
                        )

            es_by = {}
            queue = deque()
            acc = 0.0

            # ---------------- Phase A: projections ----------------
            with (
                tc.tile_pool(name="wqkv", bufs=1) as wpool,
                tc.tile_pool(name="xs", bufs=3) as xpool,
                tc.tile_pool(name="psQK", bufs=6, space="PSUM") as psQK,
                tc.tile_pool(name="psV", bufs=2, space="PSUM") as psV,
            ):
                wq = wpool.tile([128, KO, M], bf16, tag="wq")
                wk = wpool.tile([128, KO, M], bf16, tag="wk")
                wv = wpool.tile([128, KO, M], bf16, tag="wv")
                if with_bias:
                    bias = wpool.tile([1, 3, M], bf16, tag="bias")
                    ones_t = wpool.tile([128, 512], bf16, tag="ones_a")
                    ones = ones_t[0:1, :]

                def qk_bias_mm(ps, bi, h):
                    nc.tensor.matmul(
                        ps,
                        lhsT=bias[:, bi, h * HD : (h + 1) * HD],
                        rhs=ones,
                        start=False,
                        stop=True,
                    )

                def pair_dma(pair, xt_h):
                    if pair == 0:
                        # JIT startup: 2-ko X chunks interleaved with wq so
                        # the four Q accumulation chains start ~2us in and
                        # stay fed; everything phase B needs comes last
                        for g in range(8):
                            xsl = slice((g % 4) * 2, (g % 4) * 2 + 2)
                            (nc.scalar if g % 2 else nc.sync).dma_start(
                                wq[:, 2 * g : 2 * g + 2],
                                wqt_p[:, 2 * g : 2 * g + 2],
                            )
                            (nc.sync if g % 2 else nc.scalar).dma_start(
                                xt_h[g // 4][:, xsl],
                                xt_p[:, 0, 2 * g : 2 * g + 2],
                            )
                        nc.sync.dma_start(wk, wkt_p[:])
                        nc.sync.dma_start(wv, wvt_p[:])
                        nc.sync.dma_start(masks, mask_p[:])
                        nc.sync.dma_start(wot, wot_p[:])
                        nc.sync.dma_start(iden, iden_p[:])
                        if with_bias:
                            nc.sync.dma_start(bias, bias_p[:])
                            nc.sync.dma_start(ones_t, ones_p[:])
                    else:
                        for half in range(2):
                            nc.sync.dma_start(
                                xt_h[half],
                                xt_p[:, pair, half * 8 : half * 8 + 8],
                            )

                def pair_ops(pair, xt_h):
                    """Projection compute for one 1024-token pair as a list
                    of (cost_ns, fn) ops at single-ko granularity."""
                    b = pair // 2
                    state = {}

                    def xt_at(ko, sub):
                        return xt_h[ko // 8][
                            :, ko % 8, sub * 512 : (sub + 1) * 512
                        ]

                    ops = []
                    for sub in range(2):
                        s0 = (pair * 1024 + sub * 512) % S
                        for wt, dst, bi in ((wq, qt_res, 0), (wk, kt_res, 1)):
                            for h in range(HPC):
                                for ko in range(KO):
                                    def fn(
                                        sub=sub, s0=s0, wt=wt, dst=dst,
                                        bi=bi, h=h, ko=ko,
                                    ):
                                        key = (sub, bi, h)
                                        if ko == 0:
                                            state[key] = psQK.tile(
                                                [128, 512], f32,
                                                tag="qk", name="psqk",
                                            )
                                        ps = state[key]
                                        nc.tensor.matmul(
                                            ps,
                                            lhsT=wt[:, ko, h * HD : (h + 1) * HD],
                                            rhs=xt_at(ko, sub),
                                            start=(ko == 0),
                                            stop=(ko == KO - 1)
                                            and not with_bias,
                                        )
                                        if ko == KO - 1:
                                            if with_bias:
                                                qk_bias_mm(ps, bi, h)
                                            nc.vector.tensor_copy(
                                                dst[:, b, h, s0 : s0 + 512], ps
                                            )
                                    ops.append((213, fn))
                        for tsub in range(4):
                            for ko in range(KO):
                                def fn(sub=sub, s0=s0, tsub=tsub, ko=ko):
                                    key = ("v", sub, tsub)
                                    if ko == 0:
                                        state[key] = psV.tile(
                                            [128, M], f32, tag="v", name="psv"
                                        )
                                    ps = state[key]
                                    nc.tensor.matmul(
                                        ps,
                                        lhsT=xt_at(ko, sub)[
                                            :, tsub * 128 : (tsub + 1) * 128
                                        ],
                                        rhs=wv[:, ko],
                                        start=(ko == 0),
                                        stop=(ko == KO - 1) and not with_bias,
                                    )
                                    if ko == KO - 1:
                                        if with_bias:
                                            nc.tensor.matmul(
                                                ps,
                                                lhsT=ones[:, :128],
                                                rhs=bias[:, 2],
                                                start=False,
                                                stop=True,
                                            )
                                        sc = (s0 + tsub * 128) // 128
                                        nc.vector.tensor_copy(
                                            v_res[:, b, :, sc, 0:HD],
                                            ps.rearrange(
                                                "p (h d) -> p h d", h=HPC
                                            ),
                                        )
                                ops.append((107, fn))
                    return ops

                # pair 0: special interleaved-chain startup, emitted bare
                xt_h0 = [
                    xpool.tile([128, KO // 2, 1024], bf16, tag="xt", name="xth")
                    for _ in range(2)
                ]
                pair_dma(0, xt_h0)

                def xt_at0(ko, sub):
                    return xt_h0[ko // 8][:, ko % 8, sub * 512 : (sub + 1) * 512]

                chains = [(h, sub) for h in range(HPC) for sub in range(2)]
                ps_q = {
                    c: psQK.tile([128, 512], f32, tag="qk", name="psq")
                    for c in chains
                }
                for ko in range(KO):
                    for h, sub in chains:
                        nc.tensor.matmul(
                            ps_q[(h, sub)],
                            lhsT=wq[:, ko, h * HD : (h + 1) * HD],
                            rhs=xt_at0(ko, sub),
                            start=(ko == 0),
                            stop=(ko == KO - 1) and not with_bias,
                        )
                for h, sub in chains:
                    if with_bias:
                        qk_bias_mm(ps_q[(h, sub)], 0, h)
                    nc.vector.tensor_copy(
                        qt_res[:, 0, h, sub * 512 : sub * 512 + 512],
                        ps_q[(h, sub)],
                    )
                # pair 0 K + V (reuse pair_ops minus the already-done Q):
                # per-sub layout is Q(2x16) K(2x16) V(4x16) = 128 ops
                ops0 = pair_ops(0, xt_h0)
                for _, fn in ops0[32:128]:      # K + V of sub 0
                    fn()
                for _, fn in ops0[160:256]:     # K + V of sub 1
                    fn()

                # pair 1: emitted bare
                xt_h1 = [
                    xpool.tile([128, KO // 2, 1024], bf16, tag="xt", name="xth")
                    for _ in range(2)
                ]
                pair_dma(1, xt_h1)
                for _, fn in pair_ops(1, xt_h1):
                    fn()

                # batch 0's Q/K/V are ready: weave groups 0+1's scores into
                # batch 1's projection compute (exps run during phase A)
                queue_scores(queue, psQK, "qk", 0, es_by)
                queue_scores(queue, psQK, "qk", 1, es_by)
                for pair in (2, 3):
                    xt_h = [
                        xpool.tile(
                            [128, KO // 2, 1024], bf16, tag="xt", name="xth"
                        )
                        for _ in range(2)
                    ]
                    pair_dma(pair, xt_h)
                    acc = weave(pair_ops(pair, xt_h), queue, acc)

            # ------------- Phase B + C: attention + out projection -------------
            with (
                tc.tile_pool(name="ctxn", bufs=12) as ctxn,
                tc.tile_pool(name="recp", bufs=12) as recp,
                tc.tile_pool(name="ctxT", bufs=2) as ctxTp,
                tc.tile_pool(name="ob", bufs=3) as obp,
                tc.tile_pool(name="psS", bufs=2, space="PSUM") as psS,
                tc.tile_pool(name="psC", bufs=3, space="PSUM") as psC,
                tc.tile_pool(name="psT", bufs=1, space="PSUM") as psT,
                tc.tile_pool(name="psO", bufs=2, space="PSUM") as psO,
            ):
                def av_ops(b, qb, es, cns_out, tc_i=None):
                    """One op per k-chunk j of each 128-query chunk i; the
                    closing op of each i-chunk adds the DVE rec+normalize
                    (plus, for the last group, its transposes via tc_i)."""
                    ops = []
                    state = {}
                    for i in range(4):
                        qi = 4 * qb + i
                        for j in range(qi + 1):
                            def fn(i=i, j=j, qi=qi):
                                if j == 0:
                                    state[i] = [
                                        psC.tile(
                                            [128, 512], f32, tag="c", name="psc"
                                        )
                                        for _ in range(HPC)
                                    ]
                                pscs = state[i]
                                for h in range(HPC):
                                    nc.tensor.matmul(
                                        pscs[h][:, 0:HD1],
                                        lhsT=es[2 * j + h][
                                            :, i * 128 : (i + 1) * 128
                                        ],
                                        rhs=v_res[:, b, h, j, :],
                                        start=(j == 0),
                                        stop=(j == qi),
                                    )
                                if j == qi:
                                    cns_pair = []
                                    for h in range(HPC):
                                        rec = recp.tile(
                                            [128, 1], f32, tag="r", name="rec"
                                        )
                                        nc.vector.reciprocal(
                                            rec, pscs[h][:, HD:HD1]
                                        )
                                        cn = ctxn.tile(
                                            [128, 128], bf16, tag="cn", name="cn"
                                        )
                                        nc.vector.tensor_scalar_mul(
                                            cn, pscs[h][:, 0:HD], rec
                                        )
                                        cns_pair.append(cn)
                                    cns_out.extend(cns_pair)
                                    if tc_i is not None:
                                        tc_i(i, cns_pair)
                            ops.append((110, fn))
                    return ops

                def make_tc_t(ct):
                    """Per-chunk transposes for the final group, so the
                    epilogue is only the out projection."""
                    def tc_i(i, cns_pair):
                        for h in range(HPC):
                            pst = psT.tile([128, 512], bf16, tag="t", name="pst")
                            nc.tensor.transpose(pst[:, 0:128], cns_pair[h], iden)
                            nc.vector.tensor_copy(
                                ct[:, h, i * 128 : (i + 1) * 128], pst[:, 0:128]
                            )
                    return tc_i

                def tc_ops(b, qb, cns, ct, skip_T=False, alt_q=False,
                           cp=None):
                    """Transpose normalized ctx, then the out projection.
                    Output tiles are paired into one DMA per 256 rows."""
                    ops = []
                    if not skip_T:
                        for i in range(4):
                            for h in range(HPC):
                                def fn(i=i, h=h):
                                    pst = psT.tile(
                                        [128, 512], bf16, tag="t", name="pst"
                                    )
                                    nc.tensor.transpose(
                                        pst[:, 0:128], cns[2 * i + h], iden
                                    )
                                    nc.vector.tensor_copy(
                                        ct[:, h, i * 128 : (i + 1) * 128],
                                        pst[:, 0:128],
                                    )
                                ops.append((110, fn))
                    state = {}
                    for oc in range(KO):
                        def fn(oc=oc):
                            pso = psO.tile([128, 512], f32, tag="o", name="pso")
                            for h in range(HPC):
                                nc.tensor.matmul(
                                    pso,
                                    lhsT=wot[:, h, oc],
                                    rhs=ct[:, h, :],
                                    start=(h == 0),
                                    stop=(h == HPC - 1),
                                )
                            def ccopy(dst, src, oc=oc):
                                if cp and cp[oc % len(cp)] == "act":
                                    nc.scalar.activation(dst, src, AF.Copy)
                                else:
                                    nc.vector.tensor_copy(dst, src)
                            if oc % 2 == 0:
                                state["ob"] = obp.tile(
                                    [128, 2, 512], f32, tag="ob", name="ob"
                                )
                                ccopy(state["ob"][:, 0], pso)
                            else:
                                ob = state["ob"]
                                ccopy(ob[:, 1], pso)
                                eng = (
                                    nc.scalar
                                    if alt_q and (oc // 2) % 2
                                    else nc.sync
                                )
                                eng.dma_start(
                                    out_p[
                                        b,
                                        (oc - 1) * 128 : (oc + 1) * 128,
                                        qb * 512 : (qb + 1) * 512,
                                    ].rearrange("(u p) s -> p u s", u=2),
                                    ob,
                                )
                        ops.append((430, fn))
                    return ops

                prev = None
                last_tc = None
                for gi, (b, qb) in enumerate(groups):
                    if gi + 2 < len(groups):
                        queue_scores(queue, psS, "s", gi + 2, es_by)
                    if prev is None:
                        pre_ops = []
                    elif gi == len(groups) - 1:
                        pre_ops = tc_ops(*prev, cp=["act", "dve"])
                    else:
                        pre_ops = tc_ops(*prev)
                    acc = weave(pre_ops, queue, acc)
                    # barrier: scores(g) must all be emitted before AV(g)
                    while queue and queue[0][0] <= gi:
                        queue.popleft()[1]()
                        acc = 0.0
                    cns = []
                    ct = ctxTp.tile([128, HPC, 512], bf16, tag="ct", name="ct")
                    last = gi == len(groups) - 1
                    tci = make_tc_t(ct) if last else None
                    acc = weave(
                        av_ops(b, qb, es_by[gi], cns, tc_i=tci), queue, acc
                    )
                    if last:
                        last_tc = (b, qb, cns, ct)
                        prev = None
                    else:
                        prev = (b, qb, cns, ct)
                    del es_by[gi]
                if prev is not None:
                    for _, fn in tc_ops(*prev):
                        fn()
                if last_tc is not None:
                    for _, fn in tc_ops(
                        *last_tc, skip_T=True, alt_q=True, cp=["act"]
                    ):
                        fn()

    nc.finalize()
    return nc


def _get_nc(with_bias=False):
    if with_bias not in _built:
        _built[with_bias] = _build(with_bias)
    return _built[with_bias]


def kernel(hidden_states, attention_mask, Wq, bq, Wk, bk, Wv, bv, Wo, bo):
    hidden_states = np.asarray(hidden_states, dtype=np.float32)
    Wq, Wk, Wv, Wo = (np.asarray(w, dtype=np.float32) for w in (Wq, Wk, Wv, Wo))
    bq, bk, bv, bo = (np.asarray(v, dtype=np.float32) for v in (bq, bk, bv, bo))

    with_bias = bool(np.any(bq) or np.any(bk) or np.any(bv))

    x = hidden_states.reshape(T, D)
    # XT[p, pair, ko, t] = x[pair*1024 + t, ko*128 + p]
    xt = np.ascontiguousarray(
        x.reshape(NPAIR, 1024, KO, 128).transpose(3, 0, 2, 1)
    ).astype(BF16)

    # causal 0/1 masks for the 4 diagonal-tile offsets:
    # mask[p, i, f] = p + 128*i <= f
    p_idx = np.arange(128)[:, None, None]
    i_idx = np.arange(4)[None, :, None]
    f_idx = np.arange(512)[None, None, :]
    mask = (p_idx + 128 * i_idx <= f_idx).astype(BF16)
    iden = np.eye(128, dtype=BF16)
    ones = np.ones((128, 512), dtype=BF16)

    in_maps = []
    for c in range(NCORES):
        rows = slice(c * M, (c + 1) * M)
        # W*T[p, ko, m] = W[rows0 + m, ko*128 + p]
        wqt = np.ascontiguousarray(
            Wq[rows, :].T.reshape(KO, 128, M).transpose(1, 0, 2)
        ).astype(BF16)
        wkt = np.ascontiguousarray(
            Wk[rows, :].T.reshape(KO, 128, M).transpose(1, 0, 2)
        ).astype(BF16)
        wvt = np.ascontiguousarray(
            Wv[rows, :].T.reshape(KO, 128, M).transpose(1, 0, 2)
        ).astype(BF16)
        # WOT[p, h, oc, j] = Wo[oc*128 + j, rows0 + h*128 + p]
        wot = np.ascontiguousarray(
            Wo[:, rows].reshape(KO, 128, HPC, 128).transpose(3, 2, 0, 1)
        ).astype(BF16)
        bias = np.stack([bq[rows], bk[rows], bv[rows]])[None].astype(BF16)
        in_maps.append(
            {
                "XT": xt,
                "WQT": wqt,
                "WKT": wkt,
                "WVT": wvt,
                "WOT": wot,
                "BIAS": np.ascontiguousarray(bias),
                "MASK": mask,
                "IDEN": iden,
                "ONES": ones,
            }
        )

    res = run_bass_kernel_spmd(_get_nc(with_bias), in_maps, list(range(NCORES)))
    out = res.results[0]["OUT"].copy()
    for c in range(1, NCORES):
        out += res.results[c]["OUT"]
    out = np.ascontiguousarray(out.transpose(0, 2, 1))
    out += bo
    return out
